# revision 7
# baseline (speedup 1.0000x reference)
"""Trainium2 Bass kernel for nn_Attend: softmax(q@k^T * scale + bias) @ v.

Shapes (full problem):
  q:         [B=2, H=8, S=2048, D=64] fp32
  k, v:      [B=2, S=2048, D=64]      fp32 (shared across heads)
  mask:      [B=2, S=2048] bool       (all ones in practice)
  attn_bias: [B=2, H=8, S=2048, S=2048] fp32
  out:       [B=2, H=8, S=2048, D=64] fp32

Sharding: 16 (b,h) pairs over 8 cores -> 2 heads per core, k/v replicated
per-b (4 cores share each b).

Per-core algorithm (fp16 compute, fp32 PSUM accumulation):
  - kT, qT built once via PE transposes into [128(zero-padded d), S] fp16;
    q pre-scaled by 1/sqrt(D). K padded to 128 so every stationary operand
    is full-height (fast-weight-load path; K=64 stationaries stall LDW).
  - S^T[j, i] per (head, 512-i-chunk, 128-j-tile): matmul(kT_tile, qT_chunk)
    into PSUM fp32; bias added by 4 matmuls using the NATURAL [i', j] bias
    block as the STATIONARY operand and a 128x128 identity as the moving
    operand (out = bias_blk.T @ I) accumulating into the same PSUM bank.
    Bias DMA + fp32->fp16 cast are prefetched one chunk ahead so the
    in-order DVE stream never blocks the PE at chunk boundaries.
  - P^T = exp(S^T - 2) via ScalarE, PSUM -> SBUF fp16, 1024 cols per
    instruction (softmax is shift-invariant; -2 keeps exp within fp16 range).
  - out^T[d, i] accumulated over j-tiles: matmul(v_aug, P^T), v_aug carrying
    a ones-column -> row 64 of out^T is the softmax denominator. PV matmuls
    are software-pipelined one j-pair behind so PE never waits on ScalarE.
  - Epilogue: small PE transposes back to [i, d], reciprocal + scale, DMA out.
"""

import sys

sys.path.insert(0, "/opt/trn_rl_repo")

from contextlib import ExitStack

import numpy as np

B, H, S, D = 2, 8, 2048, 64
NH = 2          # heads per core
N_CORES = 8
IC = S // 512   # i-chunks per head
JT = S // 128   # j-tiles
JP = JT // 2    # j-tile pairs
NCHUNK = NH * IC

_cache = {}


def _build():
    import concourse.bacc as bacc
    import concourse.tile as tile
    from concourse import masks, mybir

    f32 = mybir.dt.float32
    f16 = mybir.dt.float16
    Exp = mybir.ActivationFunctionType.Exp

    nc = bacc.Bacc("TRN2", target_bir_lowering=False, debug=False,
                   num_devices=N_CORES)
    q_ap = nc.dram_tensor("q", [NH, S, D], f32, kind="ExternalInput").ap()
    k_ap = nc.dram_tensor("k", [S, D], f32, kind="ExternalInput").ap()
    v_ap = nc.dram_tensor("v", [S, D], f32, kind="ExternalInput").ap()
    bias_ap = nc.dram_tensor("bias", [NH, S, S], f32, kind="ExternalInput").ap()
    out_ap = nc.dram_tensor("out", [NH, S, D], f32, kind="ExternalOutput").ap()

    with tile.TileContext(nc) as tc, ExitStack() as ctx:
        const_pool = ctx.enter_context(tc.tile_pool(name="const", bufs=1))
        prep_sb = ctx.enter_context(tc.tile_pool(name="prep_sb", bufs=2))
        small_ps = ctx.enter_context(
            tc.tile_pool(name="small_ps", bufs=1, space="PSUM"))
        bias_pool = ctx.enter_context(tc.tile_pool(name="bias", bufs=2))
        st_pool = ctx.enter_context(
            tc.tile_pool(name="st", bufs=3, space="PSUM"))
        pt_pool = ctx.enter_context(tc.tile_pool(name="pt", bufs=3))
        ov_pool = ctx.enter_context(
            tc.tile_pool(name="ov", bufs=1, space="PSUM"))
        epi_sb = ctx.enter_context(tc.tile_pool(name="epi_sb", bufs=2))

        ident = const_pool.tile([128, 128], f16)
        masks.make_identity(nc, ident[:])
        ident32 = const_pool.tile([128, 128], f32)
        masks.make_identity(nc, ident32[:])
        shift = const_pool.tile([128, 1], f32)
        nc.vector.memset(shift[:], -2.0)

        # bias load (DMA + cast) for chunk idx -- prefetched one chunk ahead
        def load_bias(idx):
            h, c = divmod(idx, IC)
            src = bias_ap[h, c * 512:(c + 1) * 512, :].rearrange(
                "(s p) j -> p s j", p=128)
            bias_f = bias_pool.tile([128, 4, S], f32, tag="biasf",
                                    name=f"bias_f{idx}")
            bias_t = bias_pool.tile([128, 4, S], f16, tag="biast",
                                    name=f"bias_t{idx}")
            for s in range(4):
                nc.sync.dma_start(bias_f[:, s, :], src[:, s, :])
                nc.vector.tensor_copy(bias_t[:, s, :], bias_f[:, s, :])
            return bias_t

        # chunk 0 bias starts before anything else (longest pole at ramp)
        bias_cur = load_bias(0)

        # ---- prep: kT/qT [128(pad), S] f16 (q scaled), v_aug [128, 16*65]
        kT = const_pool.tile([128, S], f16)
        qT = const_pool.tile([128, NH * S], f16)
        v_aug = const_pool.tile([128, JT * 65], f16)
        nc.vector.memset(kT[:], 0.0)
        nc.vector.memset(qT[:], 0.0)
        nc.vector.memset(v_aug[:], 1.0)

        kv_f = prep_sb.tile([128, 2, JT, 64], f32, tag="kv", name="kv_f")
        nc.sync.dma_start(kv_f[:, 0], k_ap.rearrange("(t p) d -> p t d", p=128))
        nc.sync.dma_start(kv_f[:, 1], v_ap.rearrange("(t p) d -> p t d", p=128))
        q_f = prep_sb.tile([128, NH, JT, 64], f32, tag="qf", name="q_f")
        for h in range(NH):
            nc.sync.dma_start(
                q_f[:, h], q_ap[h].rearrange("(t p) d -> p t d", p=128))

        k16 = prep_sb.tile([128, JT, 64], f16, tag="k16", name="k16")
        nc.vector.tensor_copy(k16[:], kv_f[:, 0])
        q16 = prep_sb.tile([128, NH, JT, 64], f16, tag="q16", name="q16")
        for h in range(NH):
            nc.vector.tensor_scalar_mul(q16[:, h], q_f[:, h],
                                        float(D) ** -0.5)
        for jt in range(JT):
            nc.scalar.copy(v_aug[:, jt * 65:jt * 65 + 64], kv_f[:, 1, jt])
        # transposes: 4 tiles into one psum bank, then one copy out (ScalarE)
        for g in range(JT // 4):
            p = small_ps.tile([64, 512], f16, tag="sm", name=f"ktp{g}")
            for u in range(4):
                nc.tensor.matmul(p[:, u * 128:(u + 1) * 128],
                                 k16[:, g * 4 + u], ident[:],
                                 is_transpose=True, start=True, stop=True)
            nc.scalar.copy(kT[0:64, g * 512:(g + 1) * 512], p[:])
        for h in range(NH):
            for g in range(JT // 4):
                p = small_ps.tile([64, 512], f16, tag="sm", name=f"qtp{h}_{g}")
                for u in range(4):
                    nc.tensor.matmul(p[:, u * 128:(u + 1) * 128],
                                     q16[:, h, g * 4 + u], ident[:],
                                     is_transpose=True, start=True, stop=True)
                nc.scalar.copy(
                    qT[0:64, h * S + g * 512: h * S + (g + 1) * 512], p[:])

        # ---- main: per (head, i-chunk of 512)
        for idx in range(NCHUNK):
            h, c = divmod(idx, IC)
            bias_t = bias_cur
            if idx + 1 < NCHUNK:
                bias_cur = load_bias(idx + 1)   # prefetch next chunk
            ov = ov_pool.tile([65, 512], f32)
            prev_pt = None
            for p in range(JP):
                st = st_pool.tile([128, 1024], f32)
                for u in range(2):
                    jt = 2 * p + u
                    nc.tensor.matmul(
                        st[:, u * 512:(u + 1) * 512],
                        kT[:, jt * 128:(jt + 1) * 128],
                        qT[:, h * S + c * 512: h * S + (c + 1) * 512],
                        start=True, stop=False, skip_group_check=True)
                    for s in range(4):
                        nc.tensor.matmul(
                            st[:, u * 512 + s * 128: u * 512 + (s + 1) * 128],
                            bias_t[:, s, jt * 128:(jt + 1) * 128],
                            ident[:], start=False, stop=(s == 3),
                            skip_group_check=True)
                if prev_pt is not None:
                    for u in range(2):
                        jt = 2 * (p - 1) + u
                        nc.tensor.matmul(
                            ov[:], v_aug[:, jt * 65: jt * 65 + 65],
                            prev_pt[:, u * 512:(u + 1) * 512],
                            start=(jt == 0), stop=False,
                            skip_group_check=True)
                pt = pt_pool.tile([128, 1024], f16)
                nc.scalar.activation(pt[:], st[:], Exp, bias=shift[:])
                prev_pt = pt
            for u in range(2):
                jt = 2 * (JP - 1) + u
                nc.tensor.matmul(
                    ov[:], v_aug[:, jt * 65: jt * 65 + 65],
                    prev_pt[:, u * 512:(u + 1) * 512],
                    start=False, stop=(u == 1), skip_group_check=True)
            # epilogue: out rows = ov[:64, :] / ov[64, :]
            ovs = epi_sb.tile([65, 512], f32, tag="ovs")
            nc.scalar.copy(ovs[:], ov[:])
            res = epi_sb.tile([128, 4, 64], f32, tag="res")
            for s in range(4):
                tp = small_ps.tile([128, 65], f32, tag="sm")
                nc.tensor.matmul(tp[:], ovs[:, s * 128:(s + 1) * 128],
                                 ident32[:65, :65], is_transpose=True,
                                 start=True, stop=True)
                rec = epi_sb.tile([128, 1], f32, tag="rec")
                nc.vector.reciprocal(rec[:], tp[:, 64:65])
                nc.vector.tensor_scalar_mul(res[:, s, :], tp[:, 0:64],
                                            rec[:])
            nc.sync.dma_start(
                out_ap[h, c * 512:(c + 1) * 512, :].rearrange(
                    "(s p) d -> p s d", p=128), res[:])

    nc.compile()
    return nc


def kernel(q, k, v, mask, attn_bias):
    from concourse.bass_utils import run_bass_kernel_spmd

    q = np.ascontiguousarray(np.asarray(q, dtype=np.float32))
    k = np.ascontiguousarray(np.asarray(k, dtype=np.float32))
    v = np.ascontiguousarray(np.asarray(v, dtype=np.float32))
    mask = np.asarray(mask)
    attn_bias = np.asarray(attn_bias, dtype=np.float32)

    if not mask.all():
        attn_bias = np.where(mask[:, None, None, :], attn_bias,
                             np.float32(-3.0e38)).astype(np.float32)

    if "nc" not in _cache:
        _cache["nc"] = _build()
    nc = _cache["nc"]

    in_maps = []
    for c in range(N_CORES):
        b = c // 4
        h0 = NH * (c % 4)
        in_maps.append({
            "q": np.ascontiguousarray(q[b, h0:h0 + NH]),
            "k": k[b],
            "v": v[b],
            "bias": np.ascontiguousarray(attn_bias[b, h0:h0 + NH]),
        })
    res = run_bass_kernel_spmd(nc, in_maps, core_ids=list(range(N_CORES)))
    out = np.empty((B, H, S, D), dtype=np.float32)
    for c in range(N_CORES):
        b = c // 4
        h0 = NH * (c % 4)
        out[b, h0:h0 + NH] = res.results[c]["out"]
    return out


# revision 8
# speedup vs baseline: 1.0014x; 1.0014x over previous
"""Trainium2 Bass kernel for nn_Attend: softmax(q@k^T * scale + bias) @ v.

Shapes (full problem):
  q:         [B=2, H=8, S=2048, D=64] fp32
  k, v:      [B=2, S=2048, D=64]      fp32 (shared across heads)
  mask:      [B=2, S=2048] bool       (all ones in practice)
  attn_bias: [B=2, H=8, S=2048, S=2048] fp32
  out:       [B=2, H=8, S=2048, D=64] fp32

Sharding: 16 (b,h) pairs over 8 cores -> 2 heads per core, k/v replicated
per-b (4 cores share each b).

Per-core algorithm (fp16 compute, fp32 PSUM accumulation):
  - kT, qT built once via PE transposes into [128(zero-padded d), S] fp16;
    q pre-scaled by 1/sqrt(D). K padded to 128 so every stationary operand
    is full-height (fast-weight-load path; K=64 stationaries stall LDW).
  - S^T[j, i] per (head, 512-i-chunk, 128-j-tile): matmul(kT_tile, qT_chunk)
    into PSUM fp32; bias added by 4 matmuls using the NATURAL [i', j] bias
    block as the STATIONARY operand and a 128x128 identity as the moving
    operand (out = bias_blk.T @ I) accumulating into the same PSUM bank.
    Bias DMA + fp32->fp16 cast are prefetched one chunk ahead so the
    in-order DVE stream never blocks the PE at chunk boundaries.
  - P^T = exp(S^T - 2) via ScalarE, PSUM -> SBUF fp16, 1024 cols per
    instruction (softmax is shift-invariant; -2 keeps exp within fp16 range).
  - out^T[d, i] accumulated over j-tiles: matmul(v_aug, P^T), v_aug carrying
    a ones-column -> row 64 of out^T is the softmax denominator. PV matmuls
    are software-pipelined one j-pair behind so PE never waits on ScalarE.
  - Epilogue: small PE transposes back to [i, d], reciprocal + scale, DMA out.
"""

import sys

sys.path.insert(0, "/opt/trn_rl_repo")

from contextlib import ExitStack

import numpy as np

B, H, S, D = 2, 8, 2048, 64
NH = 2          # heads per core
N_CORES = 8
IC = S // 512   # i-chunks per head
JT = S // 128   # j-tiles
JP = JT // 2    # j-tile pairs
NCHUNK = NH * IC

_cache = {}


def _build():
    import concourse.bacc as bacc
    import concourse.tile as tile
    from concourse import masks, mybir

    f32 = mybir.dt.float32
    f16 = mybir.dt.float16
    Exp = mybir.ActivationFunctionType.Exp

    nc = bacc.Bacc("TRN2", target_bir_lowering=False, debug=False,
                   num_devices=N_CORES)
    q_ap = nc.dram_tensor("q", [NH, S, D], f32, kind="ExternalInput").ap()
    k_ap = nc.dram_tensor("k", [S, D], f32, kind="ExternalInput").ap()
    v_ap = nc.dram_tensor("v", [S, D], f32, kind="ExternalInput").ap()
    bias_ap = nc.dram_tensor("bias", [NH, S, S], f32, kind="ExternalInput").ap()
    out_ap = nc.dram_tensor("out", [NH, S, D], f32, kind="ExternalOutput").ap()

    with tile.TileContext(nc) as tc, ExitStack() as ctx:
        const_pool = ctx.enter_context(tc.tile_pool(name="const", bufs=1))
        prep_sb = ctx.enter_context(tc.tile_pool(name="prep_sb", bufs=2))
        small_ps = ctx.enter_context(
            tc.tile_pool(name="small_ps", bufs=1, space="PSUM"))
        bias_pool = ctx.enter_context(tc.tile_pool(name="bias", bufs=2))
        st_pool = ctx.enter_context(
            tc.tile_pool(name="st", bufs=3, space="PSUM"))
        pt_pool = ctx.enter_context(tc.tile_pool(name="pt", bufs=3))
        ov_pool = ctx.enter_context(
            tc.tile_pool(name="ov", bufs=1, space="PSUM"))
        epi_sb = ctx.enter_context(tc.tile_pool(name="epi_sb", bufs=2))

        ident = const_pool.tile([128, 128], f16)
        masks.make_identity(nc, ident[:])
        ident32 = const_pool.tile([128, 128], f32)
        masks.make_identity(nc, ident32[:])
        shift = const_pool.tile([128, 1], f32)
        nc.vector.memset(shift[:], -2.0)

        # bias load for chunk idx: DMA and fp32->fp16 cast, separately
        # emittable so the in-order DVE stream is never blocked early.
        def dma_bias(idx):
            h, c = divmod(idx, IC)
            bsrc = bias_ap[h, c * 512:(c + 1) * 512, :].rearrange(
                "(s p) j -> p s j", p=128)
            bias_f = bias_pool.tile([128, 4, S], f32, tag="biasf",
                                    name=f"bias_f{idx}")
            for s in range(4):
                nc.sync.dma_start(bias_f[:, s, :], bsrc[:, s, :])
            return bias_f

        def cast_bias(idx, bias_f):
            bias_t = bias_pool.tile([128, 4, S], f16, tag="biast",
                                    name=f"bias_t{idx}")
            for s in range(4):
                nc.vector.tensor_copy(bias_t[:, s, :], bias_f[:, s, :])
            return bias_t

        # ---- prep: kT/qT [128(pad), S] f16 (q scaled), v_aug [128, 16*65]
        kT = const_pool.tile([128, S], f16)
        qT = const_pool.tile([128, NH * S], f16)
        v_aug = const_pool.tile([128, JT * 65], f16)
        nc.vector.memset(kT[:], 0.0)
        nc.vector.memset(qT[:], 0.0)
        nc.vector.memset(v_aug[:], 1.0)

        kv_f = prep_sb.tile([128, 2, JT, 64], f32, tag="kv", name="kv_f")
        q_f = prep_sb.tile([128, NH, JT, 64], f32, tag="qf", name="q_f")
        nc.sync.dma_start(kv_f[:, 0], k_ap.rearrange("(t p) d -> p t d", p=128))
        nc.sync.dma_start(
            q_f[:, 0], q_ap[0].rearrange("(t p) d -> p t d", p=128))
        bias_f0 = dma_bias(0)
        nc.sync.dma_start(kv_f[:, 1], v_ap.rearrange("(t p) d -> p t d", p=128))
        for h in range(1, NH):
            nc.sync.dma_start(
                q_f[:, h], q_ap[h].rearrange("(t p) d -> p t d", p=128))

        k16 = prep_sb.tile([128, JT, 64], f16, tag="k16", name="k16")
        nc.vector.tensor_copy(k16[:], kv_f[:, 0])
        q16 = prep_sb.tile([128, NH, JT, 64], f16, tag="q16", name="q16")
        for h in range(NH):
            nc.vector.tensor_scalar_mul(q16[:, h], q_f[:, h],
                                        float(D) ** -0.5)
        for jt in range(JT):
            nc.scalar.copy(v_aug[:, jt * 65:jt * 65 + 64], kv_f[:, 1, jt])
        # transposes: 4 tiles into one psum bank, then one copy out (ScalarE)
        for g in range(JT // 4):
            p = small_ps.tile([64, 512], f16, tag="sm", name=f"ktp{g}")
            for u in range(4):
                nc.tensor.matmul(p[:, u * 128:(u + 1) * 128],
                                 k16[:, g * 4 + u], ident[:],
                                 is_transpose=True, start=True, stop=True)
            nc.vector.tensor_copy(kT[0:64, g * 512:(g + 1) * 512], p[:])
        for h in range(NH):
            for g in range(JT // 4):
                p = small_ps.tile([64, 512], f16, tag="sm", name=f"qtp{h}_{g}")
                for u in range(4):
                    nc.tensor.matmul(p[:, u * 128:(u + 1) * 128],
                                     q16[:, h, g * 4 + u], ident[:],
                                     is_transpose=True, start=True, stop=True)
                nc.vector.tensor_copy(
                    qT[0:64, h * S + g * 512: h * S + (g + 1) * 512], p[:])

        # ---- main: per (head, i-chunk of 512)
        bias_cur = cast_bias(0, bias_f0)
        for idx in range(NCHUNK):
            h, c = divmod(idx, IC)
            bias_t = bias_cur
            if idx + 1 < NCHUNK:   # prefetch next chunk
                bias_cur = cast_bias(idx + 1, dma_bias(idx + 1))
            ov = ov_pool.tile([65, 512], f32)
            prev_pt = None
            for p in range(JP):
                st = st_pool.tile([128, 1024], f32)
                for u in range(2):
                    jt = 2 * p + u
                    nc.tensor.matmul(
                        st[:, u * 512:(u + 1) * 512],
                        kT[:, jt * 128:(jt + 1) * 128],
                        qT[:, h * S + c * 512: h * S + (c + 1) * 512],
                        start=True, stop=False, skip_group_check=True)
                    for s in range(4):
                        nc.tensor.matmul(
                            st[:, u * 512 + s * 128: u * 512 + (s + 1) * 128],
                            bias_t[:, s, jt * 128:(jt + 1) * 128],
                            ident[:], start=False, stop=(s == 3),
                            skip_group_check=True)
                if prev_pt is not None:
                    for u in range(2):
                        jt = 2 * (p - 1) + u
                        nc.tensor.matmul(
                            ov[:], v_aug[:, jt * 65: jt * 65 + 65],
                            prev_pt[:, u * 512:(u + 1) * 512],
                            start=(jt == 0), stop=False,
                            skip_group_check=True)
                pt = pt_pool.tile([128, 1024], f16)
                nc.scalar.activation(pt[:], st[:], Exp, bias=shift[:])
                prev_pt = pt
            for u in range(2):
                jt = 2 * (JP - 1) + u
                nc.tensor.matmul(
                    ov[:], v_aug[:, jt * 65: jt * 65 + 65],
                    prev_pt[:, u * 512:(u + 1) * 512],
                    start=False, stop=(u == 1), skip_group_check=True)
            # epilogue: out rows = ov[:64, :] / ov[64, :]
            ovs = epi_sb.tile([65, 512], f32, tag="ovs")
            nc.vector.tensor_copy(ovs[:], ov[:])
            res = epi_sb.tile([128, 4, 64], f32, tag="res")
            for s in range(4):
                tp = small_ps.tile([128, 65], f32, tag="sm")
                nc.tensor.matmul(tp[:], ovs[:, s * 128:(s + 1) * 128],
                                 ident32[:65, :65], is_transpose=True,
                                 start=True, stop=True)
                rec = epi_sb.tile([128, 1], f32, tag="rec")
                nc.vector.reciprocal(rec[:], tp[:, 64:65])
                nc.vector.tensor_scalar_mul(res[:, s, :], tp[:, 0:64],
                                            rec[:])
            nc.sync.dma_start(
                out_ap[h, c * 512:(c + 1) * 512, :].rearrange(
                    "(s p) d -> p s d", p=128), res[:])

    nc.compile()
    return nc


def kernel(q, k, v, mask, attn_bias):
    from concourse.bass_utils import run_bass_kernel_spmd

    q = np.ascontiguousarray(np.asarray(q, dtype=np.float32))
    k = np.ascontiguousarray(np.asarray(k, dtype=np.float32))
    v = np.ascontiguousarray(np.asarray(v, dtype=np.float32))
    mask = np.asarray(mask)
    attn_bias = np.asarray(attn_bias, dtype=np.float32)

    if not mask.all():
        attn_bias = np.where(mask[:, None, None, :], attn_bias,
                             np.float32(-3.0e38)).astype(np.float32)

    if "nc" not in _cache:
        _cache["nc"] = _build()
    nc = _cache["nc"]

    in_maps = []
    for c in range(N_CORES):
        b = c // 4
        h0 = NH * (c % 4)
        in_maps.append({
            "q": np.ascontiguousarray(q[b, h0:h0 + NH]),
            "k": k[b],
            "v": v[b],
            "bias": np.ascontiguousarray(attn_bias[b, h0:h0 + NH]),
        })
    res = run_bass_kernel_spmd(nc, in_maps, core_ids=list(range(N_CORES)))
    out = np.empty((B, H, S, D), dtype=np.float32)
    for c in range(N_CORES):
        b = c // 4
        h0 = NH * (c % 4)
        out[b, h0:h0 + NH] = res.results[c]["out"]
    return out
